# revision 11
# baseline (speedup 1.0000x reference)
"""DGAT head (single attention head GAT) on 8 Trainium2 NeuronCores.

Strategy (row-sharded attention, per the sharding hint):
  - each core owns N/8 = 1024 query rows i of the [N, N] attention matrix,
  - adj is transposed, biased (+b/a), fp16-cast and PRE-TILED on the host
    into a partition-major layout adjP[p, jb*NR + i] = adj[i, jb*128+p] + c
    so each group DMA is one large per-partition-contiguous transfer and
    j (the softmax/contraction axis) lands on SBUF partitions.  That layout
    makes the softmax row-sum a matmul with a ones-column (TensorE
    contracts over partitions) and feeds the final attn @ h matmul
    directly -- no on-chip transpose.
  - h (= x @ w), hl, hr (= h @ a1/a2) are tiny (0.1% of flops) and are
    precomputed on the host, replicated to all cores.
  - masking (adj == 0 -> -9e15): softmax is computed unmasked on device;
    the host computes the exact correction for the (rare) masked entries
    and the device adds it to the PSUM accumulator before normalizing.
  - lrelu(a*adj + b): when a*adj+b >= 0 over the whole input (true for
    adj ~ U[0,1), a=b=1) lrelu is the identity affine, and the whole
    pre-exp elementwise chain collapses into ONE fused DVE op per tile:
        m = (hl_bcast + hr_j) * (adjT + b/a)
    via scalar_tensor_tensor with a per-partition scalar hr_j, fp16
    in/out (2x DVE rate).  A general path (explicit lrelu) is kept as
    fallback.

Per-core main loop (NG iterations of SUB j-blocks of 128):
  DMA [128, SUB*1024] fp16 adjP tile (one contiguous transfer)
  -> DVE m = (hlb + hr_j)*adjP per j-block (scalar_tensor_tensor, 2x)
  -> ACT x = exp(m - B) over the whole tile (the engine floor: 1
     elem/lane/cycle)
  -> PE psum[65, 1024] += [h|1]^T_j @ x.
Epilogue: += correction, reciprocal of row sums, PE-broadcast of the
reciprocal across partitions, multiply, elu, DMA out.
"""

import numpy as np
import ml_dtypes

N = 8192
D_IN = 128
D_OUT = 64
DP1 = D_OUT + 1
M_CORES = 8
NR = N // M_CORES  # 1024 query rows per core
SUB = 8            # j-blocks of 128 per main-loop iteration
NG = (N // 128) // SUB  # main-loop iterations
JB = N // 128      # 64 j-blocks
HALF = 4           # s-blocks per exp/tensor_mul chunk (ACT granularity)
POOL_EADD = True   # offload odd-s e-adds to GPSIMD
NEG_SLOPE = 0.2

BF16 = ml_dtypes.bfloat16


def _lrelu_scalar(t: float) -> float:
    return t if t >= 0.0 else NEG_SLOPE * t


def _split_waits(nc, max_waits: int = 1):
    """This walrus build rejects instructions carrying more than ~2 sync
    waits.  Move excess waits onto same-engine NoOps inserted just before
    the over-limit instruction (the engine blocks on the NoOp's waits
    first, then issues the real instruction -- semantically identical)."""
    import concourse.mybir as mybir

    cnt = 0
    for fn in nc.m.functions:
        for bb in fn.blocks:
            out = []
            for inst in bb.instructions:
                si = inst.sync_info
                if si is not None and si.on_wait and len(si.on_wait) > max_waits:
                    waits = list(si.on_wait)
                    head, keep = waits[:-max_waits], waits[-max_waits:]
                    for i in range(0, len(head), max_waits):
                        nop = mybir.InstNoOp(
                            name=f"I-wsplit-{cnt}", engine=inst.engine
                        )
                        cnt += 1
                        nop.sync_info = mybir.SyncInfo(
                            on_wait=head[i : i + max_waits], on_update=[]
                        )
                        out.append(nop)
                    inst.sync_info = mybir.SyncInfo(
                        on_wait=keep, on_update=list(si.on_update or [])
                    )
                out.append(inst)
            bb.instructions[:] = out
    return nc


def build_nc(a: float, b: float, mode: str, exp_bias: float, reps: int = 1,
             loop: bool = False):
    """Build the SPMD Bass program (same program for all 8 cores).

    mode: 'affine'  -> lrelu(a*adj+b) == a*adj+b elementwise (host-checked);
                       c = b/a is folded into adjP on the host and e is
                       pre-scaled by a: m = (hlb + hr_j) * adjP.
          'const'   -> a == 0: host sets adjP = 1, lrelu(b) folded into e.
          'general' -> explicit lrelu via max(v, NEG_SLOPE*v); adjP = adj.T.
    exp_bias: softmax computes exp(m - exp_bias) on device; the host uses
    the same bias in the mask correction.  Cancels in normalization.
    """
    import concourse.bass as bass
    import concourse.mybir as mybir
    import concourse.tile as tile
    from concourse.vector_clock import ScopedClock
    from contextlib import ExitStack

    # Walrus's CTRL lowering rejects >2 sync waits on one instruction; the
    # stock TileContext tail drain collects one wait per logical processor.
    # Spread them across one nop each instead.
    def _drain_and_barrier(self, tick_clock, wait_clock):
        nc = self.nc
        vc = tick_clock.global_clock
        for proc in range(len(vc)):
            t = vc[proc]
            if t > 0:
                sc = ScopedClock()
                sc.require_at_least(None, proc, t)
                nop = nc.sync.nop()
                wait_clock.add_sem_waits(nop.ins, sc)
        nc.sync.drain()
        nc.all_engine_barrier()
        assert self.sems is not None
        popped = nc._tile_sem_poison_stack.pop()
        assert popped is self._sem_poison
        nc.clear_and_free_semaphores(list(self.sems.allocated().values()))
        nc.all_engine_barrier()

    tile.TileContext._drain_and_barrier = _drain_and_barrier

    dt = mybir.dt
    AF = mybir.ActivationFunctionType
    OP = mybir.AluOpType

    nc = bass.Bass()
    adjP = nc.dram_tensor("adjP", [128, JB * NR], dt.float16, kind="ExternalInput")
    rhs = nc.dram_tensor("rhs", [128, JB * DP1], dt.bfloat16, kind="ExternalInput")
    hlb = nc.dram_tensor("hlb", [128, NR], dt.float16, kind="ExternalInput")
    hrc = nc.dram_tensor("hrc", [128, JB], dt.float32, kind="ExternalInput")
    corrT = nc.dram_tensor("corrT", [DP1, NR], dt.float32, kind="ExternalInput")
    outT = nc.dram_tensor("outT", [D_OUT, NR], dt.float32, kind="ExternalOutput")

    with tile.TileContext(nc) as tc, ExitStack() as ctx:
        consts = ctx.enter_context(tc.tile_pool(name="consts", bufs=1))
        adjp = ctx.enter_context(tc.tile_pool(name="adjp", bufs=3))
        ep = ctx.enter_context(tc.tile_pool(name="ep", bufs=2))
        mp = ctx.enter_context(tc.tile_pool(name="mp", bufs=2))
        xp = ctx.enter_context(tc.tile_pool(name="xp", bufs=2))
        psum = ctx.enter_context(tc.tile_pool(name="psum", bufs=2, space="PSUM"))
        psb = ctx.enter_context(tc.tile_pool(name="psb", bufs=2, space="PSUM"))
        epi = ctx.enter_context(tc.tile_pool(name="epi", bufs=1))

        # ---- constants ----
        # tiny ones first so the first e-adds can start right after adj g0
        hlb_sb = consts.tile([128, NR], dt.float16)
        nc.sync.dma_start(hlb_sb[:], hlb[:])
        hrc_sb = consts.tile([128, JB], dt.float32)
        nc.sync.dma_start(hrc_sb[:], hrc[:])
        ones_sb = consts.tile([1, D_OUT], dt.float32)
        nc.vector.memset(ones_sb[:], 1.0)
        # rhs/corr are emitted inside the first rep after the first adj DMA
        # (rhs is pre-rearranged on the host -> contiguous 8KB descriptors)
        rhs_sb = consts.tile([128, JB * DP1], dt.bfloat16)
        corr_sb = consts.tile([DP1, NR], dt.float32)
        late_consts = [False]

        def _load_late_consts():
            nc.sync.dma_start(rhs_sb[:], rhs[:])
            nc.sync.dma_start(corr_sb[:], corrT[:])
            late_consts[0] = True

        def _rep_body():
            acc = psum.tile([DP1, NR], dt.float32)
            for g in range(NG):
                adj_sb = adjp.tile([128, SUB * NR], dt.float16)
                nc.sync.dma_start(
                    adj_sb[:], adjP[:, g * SUB * NR : (g + 1) * SUB * NR]
                )
                if not late_consts[0]:
                    _load_late_consts()
                if mode in ("affine", "const"):
                    # e = hl_i + hr_j per j-block (tensor_scalar, 4x DVE rate;
                    # odd blocks on the otherwise-idle GPSIMD), then one fp16
                    # tensor_tensor m = e * adjP per half-group (2x DVE rate).
                    # scalar_tensor_tensor would fuse these but is 1x-rate.
                    e_sb = ep.tile([128, SUB * NR], dt.float16)
                    m_sb = mp.tile([128, SUB * NR], dt.float16)
                    for s in range(SUB):
                        jb = SUB * g + s
                        eng = nc.gpsimd if (POOL_EADD and s % 2) else nc.vector
                        eng.tensor_scalar_add(
                            e_sb[:, s * NR : (s + 1) * NR],
                            hlb_sb[:],
                            hrc_sb[:, jb : jb + 1],
                        )
                    for hf in range(SUB // HALF):
                        sl = slice(hf * HALF * NR, (hf + 1) * HALF * NR)
                        nc.vector.tensor_mul(m_sb[:, sl], e_sb[:, sl], adj_sb[:, sl])
                else:  # general lrelu
                    v_sb = mp.tile([128, SUB * NR], dt.float16, tag="v")
                    nc.vector.tensor_scalar(
                        v_sb[:], adj_sb[:], float(a), float(b), OP.mult, OP.add
                    )
                    l_sb = mp.tile([128, SUB * NR], dt.float16, tag="l")
                    nc.vector.scalar_tensor_tensor(
                        l_sb[:], v_sb[:], NEG_SLOPE, v_sb[:], OP.mult, OP.max
                    )
                    e_sb = ep.tile([128, SUB * NR], dt.float16)
                    for s in range(SUB):
                        jb = SUB * g + s
                        nc.vector.tensor_scalar_add(
                            e_sb[:, s * NR : (s + 1) * NR],
                            hlb_sb[:],
                            hrc_sb[:, jb : jb + 1],
                        )
                    m_sb = mp.tile([128, SUB * NR], dt.float16)
                    nc.vector.tensor_mul(m_sb[:], l_sb[:], e_sb[:])
                x_sb = xp.tile([128, SUB * NR], dt.bfloat16)
                for hf in range(SUB // HALF):
                    sl = slice(hf * HALF * NR, (hf + 1) * HALF * NR)
                    nc.scalar.activation(
                        x_sb[:, sl], m_sb[:, sl], AF.Exp,
                        bias=float(-exp_bias), scale=1.0,
                    )
                for s in range(SUB):
                    jb = SUB * g + s
                    lhsT = rhs_sb[:, jb * DP1 : (jb + 1) * DP1]
                    for hh in range(2):
                        nc.tensor.matmul(
                            acc[:, hh * 512 : (hh + 1) * 512],
                            lhsT,
                            x_sb[:, s * NR + hh * 512 : s * NR + (hh + 1) * 512],
                            start=(jb == 0),
                            stop=(jb == JB - 1),
                        )

            # ---- epilogue: correction, normalize, elu ----
            sT = epi.tile([DP1, NR], dt.float32)
            nc.vector.tensor_add(sT[:], acc[:], corr_sb[:])
            # 1/s as exp(-ln(s)) on ACT: Ln and Exp share a table set, and
            # ACT streams the single-partition row at 1 elem/cycle (the DVE
            # reciprocal is 8 cyc/elem -> ~8.5us on one lane).
            ls = epi.tile([1, NR], dt.float32)
            nc.scalar.activation(ls[:], sT[D_OUT : D_OUT + 1, :], AF.Ln)
            rr = epi.tile([1, NR], dt.float32)
            nc.scalar.activation(rr[:], ls[:], AF.Exp, scale=-1.0)
            # broadcast across the 64 output partitions via PE outer product
            rb = psb.tile([D_OUT, NR], dt.float32)
            for hh in range(2):
                nc.tensor.matmul(
                    rb[:, hh * 512 : (hh + 1) * 512],
                    ones_sb[:],
                    rr[:, hh * 512 : (hh + 1) * 512],
                    start=True,
                    stop=True,
                )
            hpT = epi.tile([D_OUT, NR], dt.float32)
            nc.vector.tensor_mul(hpT[:], sT[:D_OUT, :], rb[:])
            # elu(x) = relu(x) + exp(min(x,0)) - 1
            neg = epi.tile([D_OUT, NR], dt.float32)
            nc.vector.tensor_scalar_min(neg[:], hpT[:], 0.0)
            ex = epi.tile([D_OUT, NR], dt.float32)
            nc.scalar.activation(ex[:], neg[:], AF.Exp)
            rel = epi.tile([D_OUT, NR], dt.float32)
            nc.vector.tensor_scalar_max(rel[:], hpT[:], 0.0)
            ot = epi.tile([D_OUT, NR], dt.float32)
            nc.vector.scalar_tensor_tensor(
                ot[:], ex[:], -1.0, rel[:], OP.add, OP.add
            )
            nc.sync.dma_start(outT[:], ot[:])

        if loop:
            with tc.For_i(0, reps, 1):
                _rep_body()
        else:
            for _rep in range(reps):
                _rep_body()

    return _split_waits(nc)


def _host_prep(input, adj, w, a, a_coeff, b_coeff):
    """Shard/layout prep on the host.  Returns (in_maps, a, b, mode, B)."""
    x = np.asarray(input, dtype=np.float32)[0].astype(np.float64)
    adj = np.asarray(adj, dtype=np.float32)
    w64 = np.asarray(w, dtype=np.float64)
    avec = np.asarray(a, dtype=np.float64).reshape(-1)
    af = float(np.asarray(a_coeff).reshape(-1)[0])
    bf = float(np.asarray(b_coeff).reshape(-1)[0])

    h = x @ w64                      # [N, 64]
    hl = h @ avec[:D_OUT]            # [N]
    hr = h @ avec[D_OUT:]            # [N]

    amin = float(adj.min())
    amax = float(adj.max())
    t_ends = (af * amin + bf, af * amax + bf)
    tmin, tmax = min(t_ends), max(t_ends)
    if af != 0.0 and tmin >= 0.0:
        mode = "affine"
    elif af == 0.0:
        mode = "const"
    else:
        mode = "general"

    l_ends = (_lrelu_scalar(tmin), _lrelu_scalar(tmax))
    e_ends = (
        hl.min() + hr.min(),
        hl.min() + hr.max(),
        hl.max() + hr.min(),
        hl.max() + hr.max(),
    )
    m_bound = max(abs(l * e) for l in l_ends for e in e_ends)
    B = max(0.0, float(m_bound) - 60.0)

    # pre-scale folded into e (hl/hr): 'affine' needs a*e; 'const' lrelu(b)*e
    if mode == "affine":
        pre = af
    elif mode == "const":
        pre = _lrelu_scalar(bf)
    else:
        pre = 1.0
    hl_s = hl * pre
    hr_s = hr * pre

    h_bf = h.astype(np.float32).astype(BF16)
    rhs_flat = np.concatenate(
        [h_bf, np.ones((N, 1), dtype=BF16)], axis=1
    )                                # [N, 65] bf16
    # pre-rearranged to the SBUF layout rhs[p, jb*DP1 + d] = rhs_flat[jb*128+p, d]
    rhs_np = np.ascontiguousarray(
        rhs_flat.reshape(JB, 128, DP1).transpose(1, 0, 2).reshape(128, JB * DP1)
    )                                # [128, JB*65] bf16, replicated
    hrc_np = np.ascontiguousarray(
        hr_s.astype(np.float32).reshape(JB, 128).T
    )                                # [128, 64] f32, replicated

    c_fold = bf / af if mode == "affine" else 0.0

    l0 = _lrelu_scalar(bf)           # lrelu value at adj == 0
    in_maps = []
    for c in range(M_CORES):
        w0, w1 = c * NR, (c + 1) * NR
        # adjP[p, jb*NR+i] = adj[w0+i, jb*128+p] (+ b/a for affine; 1 for const)
        blk = adj[w0:w1, :]                       # [NR, N]
        if mode == "const":
            adjP_c = np.ones((128, JB * NR), dtype=np.float16)
        else:
            t = blk.reshape(NR, JB, 128)           # [i, jb, p]
            if mode == "affine" and c_fold != 0.0:
                t = t + np.float32(c_fold)
            adjP_c = np.ascontiguousarray(
                t.transpose(2, 1, 0).reshape(128, JB * NR).astype(np.float16)
            )
        hlw = hl_s[w0:w1].astype(np.float32).astype(np.float16)
        hlb_c = np.ascontiguousarray(np.broadcast_to(hlw, (128, NR)))
        # exact mask correction for adj == 0 entries in this core's rows
        corr = np.zeros((DP1, NR), dtype=np.float64)
        zi, zj = np.nonzero(blk == 0.0)
        if len(zi):
            mz = l0 * (hl[w0 + zi] + hr[zj])
            ev = np.exp(mz - B)
            acc_u = np.zeros((NR, D_OUT), dtype=np.float64)
            np.add.at(acc_u, zi, ev[:, None] * h[zj])
            acc_s = np.zeros(NR, dtype=np.float64)
            np.add.at(acc_s, zi, ev)
            corr[:D_OUT, :] = -acc_u.T
            corr[D_OUT, :] = -acc_s
        in_maps.append(
            {
                "adjP": adjP_c,
                "rhs": rhs_np,
                "hlb": hlb_c,
                "hrc": hrc_np,
                "corrT": corr.astype(np.float32),
            }
        )
    return in_maps, af, bf, mode, B


def kernel(input, adj, w, a, a_coeff, b_coeff):
    from concourse.bass_utils import run_bass_kernel_spmd

    in_maps, af, bf, mode, B = _host_prep(input, adj, w, a, a_coeff, b_coeff)
    nc = build_nc(af, bf, mode, B, reps=1)
    res = run_bass_kernel_spmd(nc, in_maps, list(range(M_CORES)))
    out = np.concatenate(
        [np.asarray(res.results[c]["outT"], dtype=np.float32).T for c in range(M_CORES)],
        axis=0,
    )
    return np.ascontiguousarray(out)
